# revision 27
# baseline (speedup 1.0000x reference)
"""Causal self-attention (B=4, S=2048, D=1024, H=16) on 8 Trainium2 NeuronCores.

Sharding: tensor-parallel over heads. Core c owns heads {2c, 2c+1}:
  - c_attn column slices (128 cols each of Q/K/V blocks)
  - full attention for its 2 heads (flash-style, transposed scores)
  - c_proj row slice -> full-size partial output; host sums the 8 partials.

Numerics: bf16 matmuls everywhere (fp32 PSUM accumulation); softmax skips the
max-subtraction (scores are O(1) here and the masked positions' exp underflows
to exactly 0, matching the reference's exp(-10000-max) probabilities).

Self-contained: hardcodes shapes; no sibling imports.
"""
import numpy as np
import ml_dtypes

import concourse.bass as bass
import concourse.mybir as mybir
import concourse.tile as tile
from concourse import bacc
from concourse.bass_utils import run_bass_kernel_spmd

B, S, D, H = 4, 2048, 1024, 16
HD = D // H            # 64
N_CORES = 8
HPC = H // N_CORES     # heads per core = 2
CF = HPC * HD          # feature cols per core = 128
KT = 128               # k-tile (scores partition dim)
QT = 512               # q-tile (scores free dim)
N_QT = S // QT         # 4
N_KT = S // KT         # 16
DK = D // 128          # 8 contraction tiles for QKV

F32 = mybir.dt.float32
F32R = mybir.dt.float32r
BF16 = mybir.dt.bfloat16
AF = mybir.ActivationFunctionType

_cache = {}


def _build():
    nc = bacc.Bacc()

    x_d = nc.dram_tensor("x", [B, D, S], BF16, kind="ExternalInput")
    wq_d = nc.dram_tensor("wq", [D, CF], BF16, kind="ExternalInput")
    wk_d = nc.dram_tensor("wk", [D, CF], BF16, kind="ExternalInput")
    wv_d = nc.dram_tensor("wv", [D, CF], BF16, kind="ExternalInput")
    bqkv_d = nc.dram_tensor("bqkv", [CF, 3], F32, kind="ExternalInput")
    wp_d = nc.dram_tensor("wp", [CF, D], BF16, kind="ExternalInput")
    po_d = nc.dram_tensor("po", [B, D, S], F32, kind="ExternalOutput")

    with tile.TileContext(nc) as tc:
        with (
            tc.tile_pool(name="consts", bufs=1) as cpool,
            tc.tile_pool(name="xin", bufs=4) as xpool,
            tc.tile_pool(name="qkv", bufs=4) as qkvpool,
            tc.tile_pool(name="probs", bufs=8) as ppool,
            tc.tile_pool(name="small", bufs=4) as spool,
            tc.tile_pool(name="outs", bufs=8) as opool,
            tc.tile_pool(name="ps_a", bufs=2, space="PSUM") as psA,
            tc.tile_pool(name="ps_sc", bufs=2, space="PSUM") as psSC,
            tc.tile_pool(name="ps_cx", bufs=2, space="PSUM") as psCX,
        ):
            # ---- constants / weights ----
            wq_t = cpool.tile([128, D], BF16, tag="wq")   # [d-tile part, DK*CF free]
            wk_t = cpool.tile([128, D], BF16, tag="wk")
            wv_t = cpool.tile([128, D], BF16, tag="wv")
            for w_t, w_d in ((wq_t, wq_d), (wk_t, wk_d), (wv_t, wv_d)):
                nc.sync.dma_start(
                    w_t[:].rearrange("p (ki f) -> p ki f", ki=DK),
                    w_d[:].rearrange("(ki p) f -> p ki f", ki=DK),
                )
            bqkv_t = cpool.tile([CF, 3], F32, tag="bqkv")
            nc.sync.dma_start(bqkv_t[:], bqkv_d[:])
            wp_t = cpool.tile([CF, D], BF16, tag="wp_t")
            nc.sync.dma_start(wp_t[:], wp_d[:])

            ident = cpool.tile([128, 128], BF16, tag="ident")
            nc.gpsimd.memset(ident[:], 0.0)
            nc.gpsimd.affine_select(
                out=ident[:], in_=ident[:],
                pattern=[[-1, 128]], compare_op=mybir.AluOpType.not_equal,
                fill=1.0, base=0, channel_multiplier=1,
            )
            ones_t = cpool.tile([1, HD], BF16, tag="ones")
            nc.gpsimd.memset(ones_t[:], 1.0)
            ones16 = cpool.tile([128, N_KT], BF16, tag="ones16")
            nc.gpsimd.memset(ones16[:], 1.0)

            for b in range(B):
                # ---- QKV projection (streams X^T chunks via DMA transpose) ----
                qt_t = qkvpool.tile([128, S], BF16, tag="qt")   # Q^T (2 heads stacked)
                kt_t = qkvpool.tile([128, S], BF16, tag="kt")   # K^T
                vt_t = qkvpool.tile([128, S], BF16, tag="vt")   # V^T
                for st in range(N_QT):
                    xt = xpool.tile([128, DK, QT], BF16, tag="xt")
                    nc.sync.dma_start(
                        xt[:],
                        x_d[b, :, st * QT:(st + 1) * QT].rearrange(
                            "(ki p) s -> p ki s", ki=DK
                        ),
                    )
                    for w_t, dst, bcol in ((wq_t, qt_t, 0), (wk_t, kt_t, 1), (wv_t, vt_t, 2)):
                        ps = psA.tile([128, QT], F32, tag="psA")
                        for ki in range(DK):
                            nc.tensor.matmul(
                                ps[:], w_t[:, ki * CF:(ki + 1) * CF], xt[:, ki, :],
                                start=(ki == 0), stop=(ki == DK - 1),
                            )
                        nc.vector.tensor_scalar_add(
                            dst[:, st * QT:(st + 1) * QT], ps[:], bqkv_t[:, bcol:bcol + 1],
                        )

                # ---- V_aug: per-head transpose of V^T + ones column ----
                va = [
                    qkvpool.tile([128, N_KT, HD + 1], BF16, tag=f"va{h}", name=f"va{h}")
                    for h in range(HPC)
                ]
                for h in range(HPC):
                    nc.vector.tensor_copy(va[h][:, :, HD:HD + 1], ones16[:])
                    for ct in range(N_KT):
                        tp = psA.tile([128, HD], BF16, tag="psA", name="tp")
                        nc.tensor.transpose(
                            tp[:],
                            vt_t[h * HD:(h + 1) * HD, ct * 128:(ct + 1) * 128],
                            ident[h * HD:(h + 1) * HD, h * HD:(h + 1) * HD],
                        )
                        nc.vector.tensor_copy(va[h][:, ct, 0:HD], tp[:])

                # ---- attention (transposed scores, both heads paired) ----
                ctxT = qkvpool.tile([128, S], BF16, tag="ctxT")
                for qi in range(N_QT):
                    n_k = (qi + 1) * (QT // KT)
                    qs = slice(qi * QT, (qi + 1) * QT)
                    slabs = []
                    offs = []
                    for ki in range(n_k):
                        ks = slice(ki * KT, (ki + 1) * KT)
                        # causal trim: columns below q = ki*KT have no valid k
                        off = max(0, ki * KT - qi * QT)
                        offs.append(off)
                        # two head matmuls on distinct row groups run concurrently
                        sp = psSC.tile([128, HPC, QT], F32, tag="sc", name="sp")
                        for h in range(HPC):
                            hs = slice(h * HD, (h + 1) * HD)
                            nc.tensor.matmul(
                                sp[:, h, off:], kt_t[hs, ks],
                                qt_t[hs, qi * QT + off:(qi + 1) * QT],
                                start=True, stop=True,
                            )
                        pe = ppool.tile([128, HPC, QT], BF16, tag="pe", name="pe")
                        nc.scalar.activation(pe[:, :, off:], sp[:, :, off:],
                                             AF.Exp, bias=0.0, scale=0.125)
                        if ki * KT >= qi * QT:
                            # mask the 128-wide diagonal strip: keep f' >= p
                            nc.gpsimd.affine_select(
                                out=pe[:, :, off:off + KT], in_=pe[:, :, off:off + KT],
                                pattern=[[0, HPC], [1, KT]],
                                compare_op=mybir.AluOpType.is_ge,
                                fill=0.0, base=0, channel_multiplier=-1,
                            )
                        slabs.append(pe)
                    for h in range(HPC):
                        hs = slice(h * HD, (h + 1) * HD)
                        cx = psCX.tile([HD + 1, QT], F32, tag="cx", name="cx")
                        for ki in range(n_k):
                            off = offs[ki]
                            nc.tensor.matmul(
                                cx[:, off:], va[h][:, ki, :], slabs[ki][:, h, off:],
                                start=(ki == 0), stop=(ki == n_k - 1),
                            )
                        # softmax divide: denom -> bcast (PE) -> fast reciprocal
                        dn = spool.tile([1, QT], BF16, tag="dn", name="dn")
                        nc.vector.tensor_copy(dn[:], cx[HD:HD + 1, :])
                        bc = psA.tile([HD, QT], F32, tag="psA", name="bc")
                        nc.tensor.matmul(bc[:], ones_t[:], dn[:], start=True, stop=True)
                        rb = spool.tile([HD, QT], F32, tag="rb", name="rb")
                        nc.vector.reciprocal_approx_fast(rb[:], bc[:])
                        nc.vector.tensor_mul(ctxT[hs, qs], cx[0:HD, :], rb[:])

                    # partial c_proj for this q-tile: poT[b][:, qs] = wp.T @ ctxT[:, qs]
                    for dt_i in range(DK):
                        pp = psA.tile([128, QT], F32, tag="psA", name="pp")
                        nc.tensor.matmul(
                            pp[:], wp_t[:, dt_i * 128:(dt_i + 1) * 128],
                            ctxT[:, qs], start=True, stop=True,
                        )
                        po_t = opool.tile([128, QT], F32, tag="po", name="po_t")
                        nc.vector.tensor_copy(po_t[:], pp[:])
                        nc.sync.dma_start(
                            po_d[b, dt_i * 128:(dt_i + 1) * 128, qs],
                            po_t[:],
                        )

    nc.compile()
    return nc


def _in_maps(hidden_states, w_attn, b_attn, w_proj):
    x_bf = np.ascontiguousarray(
        np.asarray(hidden_states, dtype=np.float32).transpose(0, 2, 1)
    ).astype(ml_dtypes.bfloat16)
    w = np.asarray(w_attn, dtype=np.float32)
    ba = np.asarray(b_attn, dtype=np.float32)
    wp = np.asarray(w_proj, dtype=np.float32)
    maps = []
    for c in range(N_CORES):
        cs = slice(c * CF, (c + 1) * CF)
        maps.append({
            "x": x_bf,
            "wq": np.ascontiguousarray(w[:, 0 * D:1 * D][:, cs]).astype(ml_dtypes.bfloat16),
            "wk": np.ascontiguousarray(w[:, 1 * D:2 * D][:, cs]).astype(ml_dtypes.bfloat16),
            "wv": np.ascontiguousarray(w[:, 2 * D:3 * D][:, cs]).astype(ml_dtypes.bfloat16),
            "bqkv": np.ascontiguousarray(
                np.stack([ba[0 * D:1 * D][cs], ba[1 * D:2 * D][cs], ba[2 * D:3 * D][cs]], axis=1)
            ),
            "wp": np.ascontiguousarray(wp[cs, :]).astype(ml_dtypes.bfloat16),
        })
    return maps


def _run(hidden_states, w_attn, b_attn, w_proj, b_proj, trace=False):
    if "nc" not in _cache:
        _cache["nc"] = _build()
    nc = _cache["nc"]
    maps = _in_maps(hidden_states, w_attn, b_attn, w_proj)
    res = run_bass_kernel_spmd(nc, maps, list(range(N_CORES)), trace=trace)
    acc = np.zeros((B, D, S), dtype=np.float32)
    for c in range(N_CORES):
        acc += res.results[c]["po"]
    out = acc.transpose(0, 2, 1) + np.asarray(b_proj, dtype=np.float32)[None, None, :]
    return np.ascontiguousarray(out), res


def kernel(hidden_states, w_attn, b_attn, w_proj, b_proj):
    out, _ = _run(hidden_states, w_attn, b_attn, w_proj, b_proj, trace=False)
    return out


def _ensure_ntff_hook():
    """Register the axon NTFF profile hook if the image's antenv lacks it."""
    try:
        from antenv.axon_hooks import get_axon_ntff_profile_hook  # noqa: F401
        return
    except ImportError:
        pass
    import sys
    import types
    import antenv
    from trn_agent_boot.trn_boot import _ntff_profile_via_ctypes

    mod = types.ModuleType("antenv.axon_hooks")
    holder = [None]
    mod.set_axon_ntff_profile_hook = lambda h: holder.__setitem__(0, h)
    mod.get_axon_ntff_profile_hook = lambda: holder[0]
    sys.modules["antenv.axon_hooks"] = mod
    antenv.axon_hooks = mod
    mod.set_axon_ntff_profile_hook(_ntff_profile_via_ctypes("/opt/axon/libaxon_pjrt.so"))


def run_traced(hidden_states, w_attn, b_attn, w_proj, b_proj):
    """For test.py: returns (output, BassKernelResults with exec_time_ns/profile)."""
    _ensure_ntff_hook()
    return _run(hidden_states, w_attn, b_attn, w_proj, b_proj, trace=True)


# revision 28
# speedup vs baseline: 1.1689x; 1.1689x over previous
"""Causal self-attention (B=4, S=2048, D=1024, H=16) on 8 Trainium2 NeuronCores.

Sharding: tensor-parallel over heads. Core c owns heads {2c, 2c+1}:
  - c_attn column slices (128 cols each of Q/K/V blocks)
  - full attention for its 2 heads (flash-style, transposed scores)
  - c_proj row slice -> full-size partial output; host sums the 8 partials.

Numerics: bf16 matmuls everywhere (fp32 PSUM accumulation); softmax skips the
max-subtraction (scores are O(1) here and the masked positions' exp underflows
to exactly 0, matching the reference's exp(-10000-max) probabilities).

Self-contained: hardcodes shapes; no sibling imports.
"""
import numpy as np
import ml_dtypes

import concourse.bass as bass
import concourse.mybir as mybir
import concourse.tile as tile
from concourse import bacc
from concourse.bass_utils import run_bass_kernel_spmd

B, S, D, H = 4, 2048, 1024, 16
HD = D // H            # 64
N_CORES = 8
HPC = H // N_CORES     # heads per core = 2
CF = HPC * HD          # feature cols per core = 128
KT = 128               # k-tile (scores partition dim)
QT = 512               # q-tile (scores free dim)
N_QT = S // QT         # 4
N_KT = S // KT         # 16
DK = D // 128          # 8 contraction tiles for QKV

F32 = mybir.dt.float32
F32R = mybir.dt.float32r
BF16 = mybir.dt.bfloat16
AF = mybir.ActivationFunctionType

_cache = {}


def _build():
    nc = bacc.Bacc()

    x_d = nc.dram_tensor("x", [B, D, S], BF16, kind="ExternalInput")
    wq_d = nc.dram_tensor("wq", [D, CF], BF16, kind="ExternalInput")
    wk_d = nc.dram_tensor("wk", [D, CF], BF16, kind="ExternalInput")
    wv_d = nc.dram_tensor("wv", [D, CF], BF16, kind="ExternalInput")
    bqkv_d = nc.dram_tensor("bqkv", [CF, 3], F32, kind="ExternalInput")
    wp_d = nc.dram_tensor("wp", [CF, D], BF16, kind="ExternalInput")
    po_d = nc.dram_tensor("po", [B, D, S], F32, kind="ExternalOutput")

    with tile.TileContext(nc) as tc:
        with (
            tc.tile_pool(name="consts", bufs=1) as cpool,
            tc.tile_pool(name="xin", bufs=4) as xpool,
            tc.tile_pool(name="qkv", bufs=4) as qkvpool,
            tc.tile_pool(name="probs", bufs=8) as ppool,
            tc.tile_pool(name="small", bufs=4) as spool,
            tc.tile_pool(name="outs", bufs=8) as opool,
            tc.tile_pool(name="ps_a", bufs=2, space="PSUM") as psA,
            tc.tile_pool(name="ps_sc", bufs=2, space="PSUM") as psSC,
            tc.tile_pool(name="ps_cx", bufs=2, space="PSUM") as psCX,
        ):
            # ---- constants / weights ----
            wq_t = cpool.tile([128, D], BF16, tag="wq")   # [d-tile part, DK*CF free]
            wk_t = cpool.tile([128, D], BF16, tag="wk")
            wv_t = cpool.tile([128, D], BF16, tag="wv")
            for w_t, w_d in ((wq_t, wq_d), (wk_t, wk_d), (wv_t, wv_d)):
                nc.sync.dma_start(
                    w_t[:].rearrange("p (ki f) -> p ki f", ki=DK),
                    w_d[:].rearrange("(ki p) f -> p ki f", ki=DK),
                )
            bqkv_t = cpool.tile([CF, 3], F32, tag="bqkv")
            nc.sync.dma_start(bqkv_t[:], bqkv_d[:])
            wp_t = cpool.tile([CF, D], BF16, tag="wp_t")
            nc.sync.dma_start(wp_t[:], wp_d[:])

            ident = cpool.tile([128, 128], BF16, tag="ident")
            nc.gpsimd.memset(ident[:], 0.0)
            nc.gpsimd.affine_select(
                out=ident[:], in_=ident[:],
                pattern=[[-1, 128]], compare_op=mybir.AluOpType.not_equal,
                fill=1.0, base=0, channel_multiplier=1,
            )
            ones_t = cpool.tile([1, HD], BF16, tag="ones")
            nc.gpsimd.memset(ones_t[:], 1.0)
            ones16 = cpool.tile([128, N_KT], BF16, tag="ones16")
            nc.gpsimd.memset(ones16[:], 1.0)

            for b in range(B):
                # ---- QKV projection (streams X^T chunks via DMA transpose) ----
                qt_t = qkvpool.tile([128, S], BF16, tag="qt")   # Q^T (2 heads stacked)
                kt_t = qkvpool.tile([128, S], BF16, tag="kt")   # K^T
                vt_t = qkvpool.tile([128, S], BF16, tag="vt")   # V^T
                for st in range(N_QT):
                    xt = xpool.tile([128, DK, QT], BF16, tag="xt")
                    nc.sync.dma_start(
                        xt[:],
                        x_d[b, :, st * QT:(st + 1) * QT].rearrange(
                            "(ki p) s -> p ki s", ki=DK
                        ),
                    )
                    for w_t, dst, bcol in ((wq_t, qt_t, 0), (wk_t, kt_t, 1), (wv_t, vt_t, 2)):
                        ps = psA.tile([128, QT], F32, tag="psA")
                        for ki in range(DK):
                            nc.tensor.matmul(
                                ps[:], w_t[:, ki * CF:(ki + 1) * CF], xt[:, ki, :],
                                start=(ki == 0), stop=(ki == DK - 1),
                            )
                        nc.vector.tensor_scalar_add(
                            dst[:, st * QT:(st + 1) * QT], ps[:], bqkv_t[:, bcol:bcol + 1],
                        )

                # ---- V_aug: per-head transpose of V^T + ones column ----
                va = [
                    qkvpool.tile([128, N_KT, HD + 1], BF16, tag=f"va{h}", name=f"va{h}")
                    for h in range(HPC)
                ]
                for h in range(HPC):
                    nc.vector.tensor_copy(va[h][:, :, HD:HD + 1], ones16[:])
                    for ct in range(N_KT):
                        tp = psA.tile([128, HD], BF16, tag="psA", name="tp")
                        nc.tensor.transpose(
                            tp[:],
                            vt_t[h * HD:(h + 1) * HD, ct * 128:(ct + 1) * 128],
                            ident[h * HD:(h + 1) * HD, h * HD:(h + 1) * HD],
                        )
                        nc.vector.tensor_copy(va[h][:, ct, 0:HD], tp[:])

                # ---- attention (transposed scores, both heads paired) ----
                ctxT = qkvpool.tile([128, S], BF16, tag="ctxT")
                for qi in range(N_QT):
                    n_k = (qi + 1) * (QT // KT)
                    qs = slice(qi * QT, (qi + 1) * QT)
                    slabs = []
                    offs = []
                    for ki in range(n_k):
                        ks = slice(ki * KT, (ki + 1) * KT)
                        # causal trim: columns below q = ki*KT have no valid k
                        off = max(0, ki * KT - qi * QT)
                        offs.append(off)
                        # two head matmuls on distinct row groups run concurrently
                        sp = psSC.tile([128, HPC, QT], F32, tag="sc", name="sp")
                        for h in range(HPC):
                            hs = slice(h * HD, (h + 1) * HD)
                            nc.tensor.matmul(
                                sp[:, h, off:], kt_t[hs, ks],
                                qt_t[hs, qi * QT + off:(qi + 1) * QT],
                                start=True, stop=True,
                            )
                        pe = ppool.tile([128, HPC, QT], BF16, tag="pe", name="pe")
                        nc.scalar.activation(pe[:, :, off:], sp[:, :, off:],
                                             AF.Exp, bias=0.0, scale=0.125)
                        if ki * KT >= qi * QT:
                            # mask the 128-wide diagonal strip: keep f' >= p
                            nc.gpsimd.affine_select(
                                out=pe[:, :, off:off + KT], in_=pe[:, :, off:off + KT],
                                pattern=[[0, HPC], [1, KT]],
                                compare_op=mybir.AluOpType.is_ge,
                                fill=0.0, base=0, channel_multiplier=-1,
                            )
                        slabs.append(pe)
                    for h in range(HPC):
                        hs = slice(h * HD, (h + 1) * HD)
                        cx = psCX.tile([HD + 1, QT], F32, tag="cx", name="cx")
                        for ki in range(n_k):
                            off = offs[ki]
                            nc.tensor.matmul(
                                cx[:, off:], va[h][:, ki, :], slabs[ki][:, h, off:],
                                start=(ki == 0), stop=(ki == n_k - 1),
                            )
                        # softmax divide: denom -> bcast (PE) -> fast reciprocal
                        dn = spool.tile([1, QT], BF16, tag="dn", name="dn")
                        nc.vector.tensor_copy(dn[:], cx[HD:HD + 1, :])
                        bc = psA.tile([HD, QT], F32, tag="psA", name="bc")
                        nc.tensor.matmul(bc[:], ones_t[:], dn[:], start=True, stop=True)
                        rb = spool.tile([HD, QT], F32, tag="rb", name="rb")
                        nc.vector.reciprocal_approx_fast(rb[:], bc[:])
                        nc.vector.tensor_mul(ctxT[hs, qs], cx[0:HD, :], rb[:])

                    # partial c_proj for this q-tile: poT[b][:, qs] = wp.T @ ctxT[:, qs]
                    for dt_i in range(DK):
                        pp = psA.tile([128, QT], F32, tag="psA", name="pp")
                        nc.tensor.matmul(
                            pp[:], wp_t[:, dt_i * 128:(dt_i + 1) * 128],
                            ctxT[:, qs], start=True, stop=True,
                        )
                        po_t = opool.tile([128, QT], F32, tag="po", name="po_t")
                        if (dt_i + qi) % 2 == 0:
                            nc.scalar.copy(po_t[:], pp[:])
                        else:
                            nc.vector.tensor_copy(po_t[:], pp[:])
                        nc.sync.dma_start(
                            po_d[b, dt_i * 128:(dt_i + 1) * 128, qs],
                            po_t[:],
                        )

    nc.compile()
    return nc


def _in_maps(hidden_states, w_attn, b_attn, w_proj):
    x_bf = np.ascontiguousarray(
        np.asarray(hidden_states, dtype=np.float32).transpose(0, 2, 1)
    ).astype(ml_dtypes.bfloat16)
    w = np.asarray(w_attn, dtype=np.float32)
    ba = np.asarray(b_attn, dtype=np.float32)
    wp = np.asarray(w_proj, dtype=np.float32)
    maps = []
    for c in range(N_CORES):
        cs = slice(c * CF, (c + 1) * CF)
        maps.append({
            "x": x_bf,
            "wq": np.ascontiguousarray(w[:, 0 * D:1 * D][:, cs]).astype(ml_dtypes.bfloat16),
            "wk": np.ascontiguousarray(w[:, 1 * D:2 * D][:, cs]).astype(ml_dtypes.bfloat16),
            "wv": np.ascontiguousarray(w[:, 2 * D:3 * D][:, cs]).astype(ml_dtypes.bfloat16),
            "bqkv": np.ascontiguousarray(
                np.stack([ba[0 * D:1 * D][cs], ba[1 * D:2 * D][cs], ba[2 * D:3 * D][cs]], axis=1)
            ),
            "wp": np.ascontiguousarray(wp[cs, :]).astype(ml_dtypes.bfloat16),
        })
    return maps


def _run(hidden_states, w_attn, b_attn, w_proj, b_proj, trace=False):
    if "nc" not in _cache:
        _cache["nc"] = _build()
    nc = _cache["nc"]
    maps = _in_maps(hidden_states, w_attn, b_attn, w_proj)
    res = run_bass_kernel_spmd(nc, maps, list(range(N_CORES)), trace=trace)
    acc = np.zeros((B, D, S), dtype=np.float32)
    for c in range(N_CORES):
        acc += res.results[c]["po"]
    out = acc.transpose(0, 2, 1) + np.asarray(b_proj, dtype=np.float32)[None, None, :]
    return np.ascontiguousarray(out), res


def kernel(hidden_states, w_attn, b_attn, w_proj, b_proj):
    out, _ = _run(hidden_states, w_attn, b_attn, w_proj, b_proj, trace=False)
    return out


def _ensure_ntff_hook():
    """Register the axon NTFF profile hook if the image's antenv lacks it."""
    try:
        from antenv.axon_hooks import get_axon_ntff_profile_hook  # noqa: F401
        return
    except ImportError:
        pass
    import sys
    import types
    import antenv
    from trn_agent_boot.trn_boot import _ntff_profile_via_ctypes

    mod = types.ModuleType("antenv.axon_hooks")
    holder = [None]
    mod.set_axon_ntff_profile_hook = lambda h: holder.__setitem__(0, h)
    mod.get_axon_ntff_profile_hook = lambda: holder[0]
    sys.modules["antenv.axon_hooks"] = mod
    antenv.axon_hooks = mod
    mod.set_axon_ntff_profile_hook(_ntff_profile_via_ctypes("/opt/axon/libaxon_pjrt.so"))


def run_traced(hidden_states, w_attn, b_attn, w_proj, b_proj):
    """For test.py: returns (output, BassKernelResults with exec_time_ns/profile)."""
    _ensure_ntff_hook()
    return _run(hidden_states, w_attn, b_attn, w_proj, b_proj, trace=True)
